# revision 27
# baseline (speedup 1.0000x reference)
"""DPGN (gnn_message_passing) Trainium2 kernel — data-parallel over B on 8 cores.

Structure (see reference.py):
    pe  = PS(middle_node, point_edge)
    gen l=0..1:  pe = PS(point_node, pe);  dn = lrelu([pe[:,:, :S], dn] @ W_l^T + b_l)
    -> (dn_0, dn_1)

PS(v, ep): sim=(v_i-v_j)^2 ; h=lrelu(BN1(sim@w1)) ; h2=lrelu(BN2(h@w2)) ;
e=sigmoid(h2@w3+b3) ; epilogue(e, ep) (row normalisation).

Exploited structure:
  * e depends only on v: gen-1/2 share e(point_node) -> only two heavy passes.
  * e is SYMMETRIC in (i,j) (sim is) -> only the block-upper-triangle is
    computed (strips: row-block p covers cols 16p..159, 55% of positions).
    e_buf is zero-filled; phase C reconstructs e_full = U + U^T - U*blockdiag
    on the PE (transpose-matmul + identity-matmul accumulate).
  * BN1 stats of sim@w1 have a closed form in per-node moments of v ->
    computed exactly on host (fp64).  BN2 stats are computed on device
    (bn_stats over strips + separate diag-block stats; global = 2*T - D)
    + one tiny [128x2] AllReduce per v across the 8 cores.
  * h (f16) stays resident in SBUF for BOTH v's (113 KiB/partition);
    pass B recomputes h2 = w2 @ h on the PE and drains e with a single
    Sigmoid activation (b3 folded) straight to the packed e staging tile.

Device layout: channels on partitions.  Unit of work = "pair" = 16 rows i of
one batch: partitions 0:64 = channels of rows 16p..16p+7, 64:128 = rows
16p+8..16p+15; free dim = 8 rows x W_p cols (W_p = 160-16p).
"""

import numpy as np

import concourse.bass as bass
import concourse.bacc as bacc
import concourse.tile as tile
from concourse import mybir
from concourse.bass_utils import run_bass_kernel_spmd

F32 = mybir.dt.float32
F16 = mybir.dt.float16
AF = mybir.ActivationFunctionType
ALU = mybir.AluOpType
AX = mybir.AxisListType

B, N, C, S, G = 16, 160, 64, 80, 2
CH1 = 2 * C  # 128
BN_EPS = 1e-5
SLOPE = 0.01
N_CORES = 8
BL = B // N_CORES           # 2 local batches per core
PAIRS_PER_BL = N // 16      # 10
NTOT = B * N * N            # 409600

# strip geometry: pair p covers rows 16p..16p+15, cols 16p..159 (W = 160-16p)
WP = [N - 16 * p for p in range(PAIRS_PER_BL)]


def _rsubs(W):
    r = min(8, 512 // W)
    out = []
    i = 0
    while i < 8:
        k = min(r, 8 - i)
        out.append((i, k))
        i += k
    return out


RS = [_rsubs(W) for W in WP]                  # row-subs per pair
NSUB_BL = sum(len(r) for r in RS)             # 18 subs per (bl)
OFF = np.cumsum([0] + [8 * w for w in WP])    # strip col offset per pair
STRIP_BL = int(OFF[-1])                       # 7040 cols per (v, bl)
CNT_T = 2 * STRIP_BL                          # strip positions / partition / v
CNT_D = 2 * PAIRS_PER_BL * 8 * 16             # diag positions / partition / v

_PROG = None
TRACE = False
LAST_EXEC_NS = None
LAST_RESULTS = None


def _bn1_stats(v, w1):
    """Exact batch stats of einsum('bijc,oc->bijo', (v_i-v_j)^2, w1)."""
    Bv, Nv, _ = v.shape
    S1 = v.sum(1)
    S2 = (v ** 2).sum(1)
    P = np.einsum('bic,bid->bcd', v, v)
    Q = np.einsum('bic,bid->bcd', v ** 2, v)
    R = np.einsum('bic,bid->bcd', v ** 2, v ** 2)
    sim_sum = 2 * Nv * S2 - 2 * S1 ** 2
    M = (2 * Nv * R
         + 2 * np.einsum('bc,bd->bcd', S2, S2)
         + 4 * P ** 2
         - 4 * np.einsum('bcd,bd->bcd', Q, S1)
         - 4 * np.einsum('bdc,bc->bcd', Q, S1))
    n = Bv * Nv * Nv
    m1 = w1 @ (sim_sum.sum(0) / n)
    E2 = np.einsum('oc,cd,od->o', w1, M.sum(0) / n, w1)
    return m1, E2 - m1 ** 2


def build_program(n_cores=N_CORES, no_collective=False):
    nc = bacc.Bacc(None, target_bir_lowering=False, debug=False)

    def inp(name, shape, dt=F32):
        return nc.dram_tensor(name, list(shape), dt, kind="ExternalInput")

    VKS = ("mid", "pt")
    vshift = {vk: inp(f"vshift_{vk}", (128, BL, N)) for vk in VKS}
    vdup = {vk: inp(f"vdup_{vk}", (128, BL, N)) for vk in VKS}
    w1T = {vk: inp(f"w1T_{vk}", (128, CH1), F16) for vk in VKS}
    w2T = inp("w2T", (CH1, C), F16)
    w3pair = inp("w3pair", (CH1, 2), F16)
    ab1 = {vk: inp(f"ab1_{vk}", (CH1, 2)) for vk in VKS}
    dsum = {vk: inp(f"dsum_{vk}", (128, 2)) for vk in VKS}
    g2b2 = inp("g2b2", (C, 2))
    b3p = inp("b3p", (128, 1))
    point_edge = inp("point_edge", (BL, N, N))
    dnT0 = inp("dnT0", (S, BL, N))
    p2d_wa = inp("p2d_wa", (S, G, S))
    p2d_wb = inp("p2d_wb", (S, G, S))
    p2d_bias = inp("p2d_bias", (S, G))
    maskdiag = inp("maskdiag", (N, N))
    eyeplus = inp("eyeplus", (N, N))
    ident = inp("ident", (128, 128))
    identh = inp("identh", (128, 128), F16)
    bdmask = inp("bdmask", (N, N), F16)

    out_dn = [nc.dram_tensor(f"out{l}", [BL, N, S], F32, kind="ExternalOutput")
              for l in range(G)]

    e_buf = {vk: nc.dram_tensor(f"ebuf_{vk}", [BL, N, N], F16) for vk in VKS}
    cc_in = {vk: nc.dram_tensor(f"ccin_{vk}", [128, 2], F32) for vk in VKS}
    cc_out = {vk: nc.dram_tensor(f"ccout_{vk}", [128, 2], F32,
                                 addr_space="Shared") for vk in VKS}
    groups = [list(range(n_cores))]

    with tile.TileContext(nc) as tc, \
         tc.tile_pool(name="singles", bufs=1) as singles, \
         tc.tile_pool(name="hpool", bufs=1) as hpool, \
         tc.tile_pool(name="wp", bufs=3) as wp, \
         tc.tile_pool(name="pmm", bufs=1, space="PSUM") as pmm, \
         tc.tile_pool(name="pe_ps", bufs=1, space="PSUM") as ppe, \
         tc.tile_pool(name="pc_ps", bufs=1, space="PSUM") as pps:

        dma = nc.default_dma_engine

        def load(t, shape, dt=F32, tag=None):
            sb = singles.tile(list(shape), dt, tag=tag or t.name,
                              name=tag or t.name)
            dma.dma_start(out=sb, in_=t[tuple(slice(0, s) for s in shape)])
            return sb

        vshift_sb = {vk: load(vshift[vk], (128, BL, N)) for vk in VKS}
        vdup_sb = {vk: load(vdup[vk], (128, BL, N)) for vk in VKS}
        w1T_sb = {vk: load(w1T[vk], (128, CH1), F16) for vk in VKS}
        w2T_sb = load(w2T, (CH1, C), F16)
        w3p_sb = load(w3pair, (CH1, 2), F16)
        ab1_sb = {vk: load(ab1[vk], (CH1, 2)) for vk in VKS}
        dsum_sb = {vk: load(dsum[vk], (128, 2)) for vk in VKS}
        g2b2_sb = load(g2b2, (C, 2))
        b3_sb = load(b3p, (128, 1))
        dnT0_sb = load(dnT0, (S, BL, N))
        p2dwa_sb = load(p2d_wa, (S, G, S))
        p2dwb_sb = load(p2d_wb, (S, G, S))
        p2db_sb = load(p2d_bias, (S, G))
        ident_sb = load(ident, (128, 128))
        identh_sb = load(identh, (128, 128), F16)
        mask_sb = [load(maskdiag, (128, N), tag="mask0"),
                   singles.tile([32, N], F32, tag="mask1", name="mask1")]
        dma.dma_start(out=mask_sb[1], in_=maskdiag[128:160, :])
        eyep_sb = [load(eyeplus, (128, N), tag="eyep0"),
                   singles.tile([32, N], F32, tag="eyep1", name="eyep1")]
        dma.dma_start(out=eyep_sb[1], in_=eyeplus[128:160, :])
        bdm_sb = [load(bdmask, (128, N), F16, tag="bdm0"),
                  singles.tile([32, N], F16, tag="bdm1", name="bdm1")]
        dma.dma_start(out=bdm_sb[1], in_=bdmask[128:160, :])

        # zero-fill e_buf (one zero tile streamed out 4x)
        zsb = singles.tile([32, 800], F16, tag="zsb", name="zsb")
        nc.vector.memset(zsb, 0.0)
        for vk in VKS:
            ezv = e_buf[vk].rearrange("l (g r) c -> l g (r c)", g=32)
            for bl in range(BL):
                dma.dma_start(out=ezv[bl], in_=zsb)

        # resident h strips, f16: 128 x (4 * 7040) x 2B = 113 KiB/partition
        h_all = hpool.tile([128, 2, 2, 2, STRIP_BL], F16, tag="h_all")
        # stats: strips (T) and diag blocks (D), 18 subs per (bl)
        stats_T = {vk: singles.tile([128, 2 * PAIRS_PER_BL, 6], F32,
                                    tag=f"statsT_{vk}", name=f"statsT_{vk}")
                   for vk in VKS}

        def vki(vk):
            return 0 if vk == "mid" else 1

        # ---------------- pass A ----------------
        def pass_a(vk):
            for bl in range(BL):
                for p in range(PAIRS_PER_BL):
                    W = WP[p]
                    FW = 8 * W
                    a_sh = vshift_sb[vk][:, bl, 16 * p:16 * p + 8]
                    in0 = a_sh.unsqueeze(-1).broadcast_to([128, 8, W])
                    in1 = (vdup_sb[vk][:, bl, 16 * p:]
                           .unsqueeze(1).broadcast_to([128, 8, W]))
                    simtmp = wp.tile([128, 8 * N], F16, tag="simtmp")
                    stv = simtmp[:, :FW].rearrange("q (a b) -> q a b", b=W)
                    nc.gpsimd.tensor_sub(stv, in0, in1)
                    sim = wp.tile([128, 8 * N], F16, tag="sim")
                    nc.vector.tensor_mul(sim[:, :FW], simtmp[:, :FW],
                                         simtmp[:, :FW])

                    hAB = h_all[:, vki(vk), bl, :,
                                int(OFF[p]):int(OFF[p]) + FW]
                    for half in (0, 1):
                        rows = sim[64 * half:64 * half + 64, :]
                        for (i0, r) in RS[p]:
                            c0, c1 = i0 * W, (i0 + r) * W
                            h1 = pmm.tile([128, 512], F32, tag="h1", bufs=3)
                            nc.tensor.matmul(
                                h1[:, :c1 - c0],
                                lhsT=w1T_sb[vk][64 * half:64 * half + 64, :],
                                rhs=rows[:, c0:c1],
                                start=True, stop=True)
                            nc.scalar.activation(
                                out=hAB[half, c0:c1], in_=h1[:, :c1 - c0],
                                func=AF.Prelu,
                                bias=ab1_sb[vk][:, 1:2],
                                scale=ab1_sb[vk][:, 0:1],
                                alpha=SLOPE)

                    sidx = sum(len(RS[q]) for q in range(p)) + bl * NSUB_BL
                    for si, (i0, r) in enumerate(RS[p]):
                        c0, c1 = i0 * W, (i0 + r) * W
                        h2 = pmm.tile([128, 512], F32, tag="h2", bufs=2)
                        nc.tensor.matmul(h2[0:64, :c1 - c0], lhsT=w2T_sb,
                                         rhs=hAB[0, c0:c1],
                                         start=True, stop=True)
                        nc.tensor.matmul(h2[64:128, :c1 - c0], lhsT=w2T_sb,
                                         rhs=hAB[1, c0:c1],
                                         start=True, stop=True)
                        nc.vector.bn_stats(
                            out=stats_T[vk][:, sidx + si, :],
                            in_=h2[:, :c1 - c0])

        # ------------- stats reduce + AllReduce -------------
        # global per-partition sums over ALL positions = 2*T - D
        def reduce_and_allreduce(vk):
            def agg(stats, cnt, tag):
                mv = singles.tile([128, 2], F32, tag=f"mv{tag}_{vk}",
                                  name=f"mv{tag}_{vk}")
                nc.vector.bn_aggr(out=mv, in_=stats)
                sm = singles.tile([128, 2], F32, tag=f"sm{tag}_{vk}",
                                  name=f"sm{tag}_{vk}")
                nc.vector.tensor_scalar_mul(sm[:, 0:1], mv[:, 0:1],
                                            float(cnt))
                msq = singles.tile([128, 1], F32, tag=f"mq{tag}_{vk}",
                                   name=f"mq{tag}_{vk}")
                nc.vector.tensor_mul(msq, mv[:, 0:1], mv[:, 0:1])
                nc.vector.tensor_add(msq, msq, mv[:, 1:2])
                nc.vector.tensor_scalar_mul(sm[:, 1:2], msq, float(cnt))
                return sm
            smT = agg(stats_T[vk], CNT_T, "T")
            sums = singles.tile([128, 2], F32, tag=f"sums_{vk}",
                                name=f"sums_{vk}")
            nc.vector.tensor_scalar_mul(sums, smT, 2.0)
            nc.vector.tensor_sub(sums, sums, dsum_sb[vk])
            dma.dma_start(out=cc_in[vk][:, :], in_=sums)
            if no_collective:
                dma.dma_start(out=cc_out[vk][:, :], in_=cc_in[vk][:, :])
            else:
                nc.gpsimd.collective_compute(
                    "AllReduce", ALU.add, replica_groups=groups,
                    ins=[cc_in[vk][:, :]], outs=[cc_out[vk][:, :]])

        # ------------- alpha2 / beta2 -------------
        def compute_ab2(vk):
            sp = wp
            gs = sp.tile([128, 2], F32, tag="gs")
            dma.dma_start(out=gs, in_=cc_out[vk][:, :])
            bot = sp.tile([C, 2], F32, tag="bot")
            dma.dma_start(out=bot, in_=gs[64:128, :])
            tot = sp.tile([C, 2], F32, tag="tot")
            nc.vector.tensor_add(tot, gs[0:64, :], bot)
            mE = sp.tile([C, 2], F32, tag="mE")
            nc.vector.tensor_scalar_mul(mE, tot, 1.0 / NTOT)
            var2 = sp.tile([C, 1], F32, tag="var2")
            nc.vector.tensor_mul(var2, mE[:, 0:1], mE[:, 0:1])
            nc.vector.tensor_sub(var2, mE[:, 1:2], var2)
            xe = sp.tile([C, 1], F32, tag="xe")
            nc.vector.tensor_scalar_add(xe, var2, BN_EPS)
            sq = sp.tile([C, 1], F32, tag="sq")
            eps_t = sp.tile([C, 1], F32, tag="eps_t")
            nc.vector.memset(eps_t, 0.0)
            nc.scalar.activation(out=sq, in_=xe, func=AF.Sqrt, bias=eps_t)
            r0 = sp.tile([C, 1], F32, tag="r0")
            nc.vector.reciprocal(r0, sq)
            t1 = sp.tile([C, 1], F32, tag="t1")
            nc.vector.tensor_mul(t1, r0, r0)
            nc.vector.tensor_mul(t1, t1, xe)
            nc.vector.tensor_scalar(t1, t1, -0.5, 1.5, ALU.mult, ALU.add)
            nc.vector.tensor_mul(r0, r0, t1)
            ab2 = sp.tile([C, 2], F32, tag="ab2")
            nc.vector.tensor_mul(ab2[:, 0:1], r0, g2b2_sb[:, 0:1])
            t2 = sp.tile([C, 1], F32, tag="t2")
            nc.vector.tensor_mul(t2, mE[:, 0:1], ab2[:, 0:1])
            nc.vector.tensor_sub(ab2[:, 1:2], g2b2_sb[:, 1:2], t2)
            ab2p = singles.tile([128, 2], F32, tag=f"ab2p_{vk}",
                                name=f"ab2p_{vk}")
            dma.dma_start(out=ab2p[0:64, :], in_=ab2)
            dma.dma_start(out=ab2p[64:128, :], in_=ab2)
            return ab2p

        # ---------------- pass B ----------------
        # e strip packed in PSUM at partition base 32*si; drained with a
        # single Sigmoid (b3 folded) -> esb_all -> strided DMA to e_buf.
        def pass_b(vk, ab2p, bl):
            esb_all = wp.tile([66, PAIRS_PER_BL, 512], F16, tag="esb",
                              bufs=1)
            ebvw = e_buf[vk].rearrange("l (pp h i) j -> l pp h i j",
                                       pp=PAIRS_PER_BL, h=2, i=8)
            for p in range(PAIRS_PER_BL):
                W = WP[p]
                FW = 8 * W
                hAB = h_all[:, vki(vk), bl, :,
                            int(OFF[p]):int(OFF[p]) + FW]
                eps_t = ppe.tile([66, 512], F32, tag="e_psum")
                maxw = 0
                for si, (i0, r) in enumerate(RS[p]):
                    c0, c1 = i0 * W, (i0 + r) * W
                    h2 = pmm.tile([128, 512], F32, tag="h2", bufs=2)
                    nc.tensor.matmul(h2[0:64, :c1 - c0], lhsT=w2T_sb,
                                     rhs=hAB[0, c0:c1],
                                     start=True, stop=True)
                    nc.tensor.matmul(h2[64:128, :c1 - c0], lhsT=w2T_sb,
                                     rhs=hAB[1, c0:c1],
                                     start=True, stop=True)
                    hh = wp.tile([128, 512], F16, tag="hh", bufs=3)
                    nc.scalar.activation(
                        out=hh[:, :c1 - c0], in_=h2[:, :c1 - c0],
                        func=AF.Prelu, bias=ab2p[:, 1:2],
                        scale=ab2p[:, 0:1], alpha=SLOPE)
                    nc.tensor.matmul(eps_t[32 * si:32 * si + 2, 0:c1 - c0],
                                     lhsT=w3p_sb, rhs=hh[:, :c1 - c0],
                                     start=True, stop=True)
                    maxw = max(maxw, c1 - c0)
                nc.scalar.activation(out=esb_all[:, p, :maxw],
                                     in_=eps_t[:, :maxw], func=AF.Sigmoid,
                                     bias=b3_sb[0:66])
                for si, (i0, r) in enumerate(RS[p]):
                    src = esb_all[32 * si:32 * si + 2, p, 0:r * W].rearrange(
                        "h (a b) -> h a b", b=W)
                    nc.gpsimd.dma_start(
                        out=ebvw[bl, p, :, i0:i0 + r, 16 * p:16 * p + W],
                        in_=src)

        # ---------------- e reconstruction:  e = U + U^T - U*blockdiag ----
        def recon(vk, bl, keep):
            U0 = keep.tile([128, N], F16, tag=f"U0_{vk}_{bl}")
            dma.dma_start(out=U0, in_=e_buf[vk][bl, 0:128, :])
            U1 = keep.tile([32, N], F16, tag=f"U1_{vk}_{bl}")
            dma.dma_start(out=U1, in_=e_buf[vk][bl, 128:160, :])
            m0 = keep.tile([128, N], F16, tag="m0")
            nc.vector.tensor_mul(m0, U0, bdm_sb[0])
            m1 = keep.tile([32, N], F16, tag="m1")
            nc.vector.tensor_mul(m1, U1, bdm_sb[1])
            tpb = pps.tile([128, 2 * N], F16, tag="tpc")
            tp0 = tpb[:, 0:N]
            tp1 = tpb[0:32, N:2 * N]
            nc.tensor.matmul(tp0[:, 0:128], lhsT=U0[:, 0:128],
                             rhs=identh_sb, is_transpose=True,
                             start=True, stop=True)
            nc.tensor.matmul(tp0[:, 128:160], lhsT=U1[:, 0:128],
                             rhs=identh_sb[0:32, 0:32], is_transpose=True,
                             start=True, stop=True)
            nc.tensor.matmul(tp1[:, 0:128], lhsT=U0[:, 128:160],
                             rhs=identh_sb, is_transpose=True,
                             start=True, stop=True)
            nc.tensor.matmul(tp1[:, 128:160], lhsT=U1[:, 128:160],
                             rhs=identh_sb[0:32, 0:32], is_transpose=True,
                             start=True, stop=True)
            e0 = keep.tile([128, N], F32, tag=f"e0_{vk}_{bl}")
            nc.vector.tensor_add(e0, tp0, m0)
            e1 = keep.tile([32, N], F32, tag=f"e1_{vk}_{bl}")
            nc.vector.tensor_add(e1, tp1, m1)
            return [e0, e1]

        # ---------------- epilogue ----------------
        def epilogue(pool, e_tiles, ep_tiles, tag):
            pe_tiles = []
            for blk, pdim in ((0, 128), (1, 32)):
                e_t, ep_t = e_tiles[blk], ep_tiles[blk]
                epm = pool.tile([pdim, N], F32, tag=f"epm{blk}")
                nc.vector.tensor_mul(epm, ep_t, mask_sb[blk][:pdim, :])
                rs = pool.tile([pdim, 1], F32, tag=f"rs{blk}")
                nc.vector.reduce_sum(rs, epm, axis=AX.X)
                x = pool.tile([pdim, N], F32, tag=f"x{blk}_{tag}")
                nc.vector.tensor_mul(x, e_t, epm)
                xs = pool.tile([pdim, 1], F32, tag=f"xs{blk}")
                nc.vector.reduce_sum(xs, x, axis=AX.X)
                nc.vector.tensor_scalar_max(xs, xs, 1e-12)
                rxs = pool.tile([pdim, 1], F32, tag=f"rxs{blk}")
                nc.vector.reciprocal(rxs, xs)
                nc.vector.tensor_mul(rxs, rxs, rs)
                nc.vector.tensor_scalar_mul(x, x, rxs)
                nc.vector.tensor_add(x, x, eyep_sb[blk][:pdim, :])
                rs2 = pool.tile([pdim, 1], F32, tag=f"rs2{blk}")
                nc.vector.reduce_sum(rs2, x, axis=AX.X)
                rrs2 = pool.tile([pdim, 1], F32, tag=f"rrs2{blk}")
                nc.vector.reciprocal(rrs2, rs2)
                nc.vector.tensor_scalar_mul(x, x, rrs2)
                pe_tiles.append(x)
            return pe_tiles

        def pe_transpose(pool, psum_pool, src_ap, pdim, fdim, tag="tp"):
            ps = psum_pool.tile([128, N], F32, tag="pc")
            nc.tensor.matmul(ps[:fdim, :pdim], lhsT=src_ap,
                             rhs=ident_sb[:pdim, :pdim],
                             is_transpose=True, start=True, stop=True)
            dst = pool.tile([fdim, pdim], F32, tag=f"tps{fdim}_{pdim}")
            nc.vector.tensor_copy(dst, ps[:fdim, :pdim])
            return dst

        # ---------------- phase C ----------------
        pe1_tiles = {}

        def phase_c1(bl, pool, keep):
            e_mid = recon("mid", bl, keep)
            ep00 = keep.tile([128, N], F32, tag=f"ep00_{bl}")
            dma.dma_start(out=ep00, in_=point_edge[bl, 0:128, :])
            ep01 = keep.tile([32, N], F32, tag=f"ep01_{bl}")
            dma.dma_start(out=ep01, in_=point_edge[bl, 128:160, :])
            pe1_tiles[bl] = epilogue(pool, e_mid, [ep00, ep01], f"pe1_{bl}")

        def phase_c2(bl, pool, keep, pps):
            e_pt = recon("pt", bl, keep)
            pe2 = epilogue(pool, e_pt, pe1_tiles[bl], f"pe2_{bl}")
            pe3 = epilogue(pool, e_pt, pe2, f"pe3_{bl}")
            dn_prev = None
            for l, pe_t in ((0, pe2), (1, pe3)):
                xT = keep.tile([S, N], F32, tag=f"xT_{bl}_{l}")
                t0 = pe_transpose(pool, pps, pe_t[0][:, 0:S], 128, S)
                nc.vector.tensor_copy(xT[:, 0:128], t0)
                t1 = pe_transpose(pool, pps, pe_t[1][:, 0:S], 32, S)
                nc.vector.tensor_copy(xT[:, 128:160], t1)
                dnT = dnT0_sb[:, bl, :] if l == 0 else dn_prev
                mm = pps.tile([128, N], F32, tag="pc")
                nc.tensor.matmul(mm[:S, :], lhsT=p2dwa_sb[:, l, :], rhs=xT,
                                 start=True, stop=False)
                nc.tensor.matmul(mm[:S, :], lhsT=p2dwb_sb[:, l, :], rhs=dnT,
                                 start=False, stop=True)
                dn_new = keep.tile([S, N], F32, tag=f"dnT_{bl}_{l}")
                nc.scalar.activation(out=dn_new, in_=mm[:S, :],
                                     func=AF.Prelu,
                                     bias=p2db_sb[:, l:l + 1],
                                     alpha=SLOPE)
                dn_prev = dn_new
                o0 = pe_transpose(pool, pps, dn_new[:, 0:128], S, 128)
                dma.dma_start(out=out_dn[l][bl, 0:128, :], in_=o0)
                o1 = pe_transpose(pool, pps, dn_new[:, 128:160], S, 32)
                dma.dma_start(out=out_dn[l][bl, 128:160, :], in_=o1)

        # ---------------- schedule ----------------
        pass_a("mid")
        reduce_and_allreduce("mid")
        pass_a("pt")
        reduce_and_allreduce("pt")
        ab2p_mid = compute_ab2("mid")
        with tc.tile_pool(name="pc", bufs=2) as pool, \
             tc.tile_pool(name="pc_keep", bufs=1) as keep:
            pass_b("mid", ab2p_mid, 0)
            pass_b("mid", ab2p_mid, 1)
            phase_c1(0, pool, keep)
            phase_c1(1, pool, keep)
            ab2p_pt = compute_ab2("pt")
            pass_b("pt", ab2p_pt, 0)
            phase_c2(0, pool, keep, pps)
            pass_b("pt", ab2p_pt, 1)
            phase_c2(1, pool, keep, pps)

    nc.compile()
    return nc


def _prep_maps(middle_node, point_node, distribution_node, distribution_edge,
               point_edge, w1, g1, b1, w2, g2, b2, w3, b3, p2d_w, p2d_b,
               n_cores=N_CORES):
    f4 = np.float32
    middle_node = np.asarray(middle_node)
    point_node = np.asarray(point_node)

    def vt_pair(v_local):
        vT = np.transpose(v_local, (0, 2, 1)).astype(f4)      # [BL, C, N]
        sh = np.concatenate([vT[:, :, 8:], np.zeros((BL, C, 8), f4)], axis=2)
        vshift = np.concatenate([vT, sh], axis=1)             # [BL, 128, N]
        vdup = np.concatenate([vT, vT], axis=1)
        # -> [128, BL, N]
        return (np.ascontiguousarray(np.transpose(vshift, (1, 0, 2))),
                np.ascontiguousarray(np.transpose(vdup, (1, 0, 2))))

    def ab1_for(v):
        m1, var1 = _bn1_stats(v.astype(np.float64), np.asarray(w1, np.float64))
        a = np.asarray(g1, np.float64) / np.sqrt(var1 + BN_EPS)
        bb = np.asarray(b1, np.float64) - m1 * a
        return np.ascontiguousarray(np.stack([a, bb], axis=1).astype(f4))

    ab1_mid = ab1_for(middle_node)
    ab1_pt = ab1_for(point_node)

    w1_64 = np.asarray(w1, np.float64)
    w2_64 = np.asarray(w2, np.float64)

    def diag_sums(v_local, ab1_h):
        # exact fp64 sums of h2 = w2 @ lrelu(a1*(w1@sim)+b1) over the
        # 16x16 diagonal blocks, split per (channel, row-half) partition
        vb = np.asarray(v_local, np.float64).reshape(BL, PAIRS_PER_BL, 16,
                                                     C)
        sim = (vb[:, :, :, None, :] - vb[:, :, None, :, :]) ** 2
        h1 = sim @ w1_64.T                                   # [bl,p,i,j,128]
        a = ab1_h[:, 0].astype(np.float64)
        b = ab1_h[:, 1].astype(np.float64)
        z = a * h1 + b
        h = np.where(z >= 0, z, SLOPE * z)
        h2 = h @ w2_64.T                                     # [bl,p,i,j,64]
        out = np.zeros((128, 2), np.float32)
        for half in range(2):
            blk = h2[:, :, 8 * half:8 * half + 8, :, :]
            out[64 * half:64 * half + 64, 0] = blk.sum((0, 1, 2, 3))
            out[64 * half:64 * half + 64, 1] = (blk ** 2).sum((0, 1, 2, 3))
        return out

    def w1T_for(ab1_h):
        w1s = np.asarray(w1, np.float64) * ab1_h[:, 0:1].astype(np.float64)
        return np.ascontiguousarray(np.concatenate(
            [w1s.T, w1s.T], axis=0).astype(np.float16))
    w1T_mid_h = w1T_for(ab1_mid)
    w1T_pt_h = w1T_for(ab1_pt)
    w2T_h = np.ascontiguousarray(np.asarray(w2).T.astype(np.float16))
    w3pair_h = np.zeros((CH1, 2), np.float16)
    w3pair_h[0:C, 0] = np.asarray(w3).astype(np.float16)
    w3pair_h[C:CH1, 1] = np.asarray(w3).astype(np.float16)
    g2b2_h = np.ascontiguousarray(np.stack([np.asarray(g2), np.asarray(b2)],
                                           axis=1).astype(f4))
    b3p_h = np.full((128, 1), float(np.asarray(b3)), f4)
    pw = np.asarray(p2d_w)
    p2d_wa_h = np.ascontiguousarray(
        np.transpose(pw[:, :, 0:S], (2, 0, 1)).astype(f4))      # [S,G,S]
    p2d_wb_h = np.ascontiguousarray(
        np.transpose(pw[:, :, S:2 * S], (2, 0, 1)).astype(f4))  # [S,G,S]
    p2d_bias_h = np.ascontiguousarray(np.asarray(p2d_b).T.astype(f4))
    maskdiag_h = (1.0 - np.eye(N)).astype(f4)
    eyeplus_h = (np.eye(N) + 1e-6).astype(f4)
    ident_h = np.eye(128, dtype=f4)
    identh_h = np.eye(128, dtype=np.float16)
    bd = np.ones((N, N), np.float16)
    for p in range(N // 16):
        bd[16 * p:16 * p + 16, 16 * p:16 * p + 16] = 0.0

    maps = []
    for c in range(n_cores):
        sl = slice(c * BL, (c + 1) * BL)
        vs_m, vd_m = vt_pair(middle_node[sl])
        vs_p, vd_p = vt_pair(point_node[sl])
        ds_m = diag_sums(middle_node[sl], ab1_mid)
        ds_p = diag_sums(point_node[sl], ab1_pt)
        dnT0_h = np.ascontiguousarray(
            np.transpose(np.asarray(distribution_node)[sl], (2, 0, 1))
            .astype(f4))                                        # [S,BL,N]
        maps.append(dict(
            vshift_mid=vs_m, vdup_mid=vd_m, vshift_pt=vs_p, vdup_pt=vd_p,
            w1T_mid=w1T_mid_h, w1T_pt=w1T_pt_h, w2T=w2T_h,
            w3pair=w3pair_h,
            ab1_mid=ab1_mid, ab1_pt=ab1_pt, g2b2=g2b2_h, b3p=b3p_h,
            dsum_mid=ds_m, dsum_pt=ds_p,
            point_edge=np.ascontiguousarray(
                np.asarray(point_edge)[sl].astype(f4)),
            dnT0=dnT0_h, p2d_wa=p2d_wa_h, p2d_wb=p2d_wb_h,
            p2d_bias=p2d_bias_h,
            maskdiag=maskdiag_h, eyeplus=eyeplus_h, ident=ident_h,
            identh=identh_h, bdmask=bd,
        ))
    return maps


def kernel(**inputs):
    global _PROG, LAST_EXEC_NS, LAST_RESULTS
    if _PROG is None:
        _PROG = build_program()
    maps = _prep_maps(**inputs)
    res = run_bass_kernel_spmd(_PROG, maps, core_ids=list(range(N_CORES)),
                               trace=TRACE)
    LAST_EXEC_NS = res.exec_time_ns
    LAST_RESULTS = res
    outs = []
    for l in range(G):
        outs.append(np.concatenate([res.results[c][f"out{l}"]
                                    for c in range(N_CORES)], axis=0))
    return tuple(outs)


# revision 28
# speedup vs baseline: 1.0820x; 1.0820x over previous
"""DPGN (gnn_message_passing) Trainium2 kernel — data-parallel over B on 8 cores.

Structure (see reference.py):
    pe  = PS(middle_node, point_edge)
    gen l=0..1:  pe = PS(point_node, pe);  dn = lrelu([pe[:,:, :S], dn] @ W_l^T + b_l)
    -> (dn_0, dn_1)

PS(v, ep): sim=(v_i-v_j)^2 ; h=lrelu(BN1(sim@w1)) ; h2=lrelu(BN2(h@w2)) ;
e=sigmoid(h2@w3+b3) ; epilogue(e, ep) (row normalisation).

Exploited structure:
  * e depends only on v: gen-1/2 share e(point_node) -> only two heavy passes.
  * e is SYMMETRIC in (i,j) (sim is) -> only the block-upper-triangle is
    computed (strips: row-block p covers cols 16p..159, 55% of positions).
    e_buf is zero-filled; phase C reconstructs e_full = U + U^T - U*blockdiag
    on the PE (transpose-matmul + identity-matmul accumulate).
  * BN1 stats of sim@w1 have a closed form in per-node moments of v ->
    computed exactly on host (fp64).  BN2 stats are computed on device
    (bn_stats over strips + separate diag-block stats; global = 2*T - D)
    + one tiny [128x2] AllReduce per v across the 8 cores.
  * h (f16) stays resident in SBUF for BOTH v's (113 KiB/partition);
    pass B recomputes h2 = w2 @ h on the PE and drains e with a single
    Sigmoid activation (b3 folded) straight to the packed e staging tile.

Device layout: channels on partitions.  Unit of work = "pair" = 16 rows i of
one batch: partitions 0:64 = channels of rows 16p..16p+7, 64:128 = rows
16p+8..16p+15; free dim = 8 rows x W_p cols (W_p = 160-16p).
"""

import numpy as np

import concourse.bass as bass
import concourse.bacc as bacc
import concourse.tile as tile
from concourse import mybir
from concourse.bass_utils import run_bass_kernel_spmd

F32 = mybir.dt.float32
F16 = mybir.dt.float16
AF = mybir.ActivationFunctionType
ALU = mybir.AluOpType
AX = mybir.AxisListType

B, N, C, S, G = 16, 160, 64, 80, 2
CH1 = 2 * C  # 128
BN_EPS = 1e-5
SLOPE = 0.01
N_CORES = 8
BL = B // N_CORES           # 2 local batches per core
PAIRS_PER_BL = N // 16      # 10
NTOT = B * N * N            # 409600

# strip geometry: pair p covers rows 16p..16p+15, cols 16p..159 (W = 160-16p)
WP = [N - 16 * p for p in range(PAIRS_PER_BL)]


def _rsubs(W):
    r = min(8, 512 // W)
    out = []
    i = 0
    while i < 8:
        k = min(r, 8 - i)
        out.append((i, k))
        i += k
    return out


RS = [_rsubs(W) for W in WP]                  # row-subs per pair
NSUB_BL = sum(len(r) for r in RS)             # 18 subs per (bl)
OFF = np.cumsum([0] + [8 * w for w in WP])    # strip col offset per pair
STRIP_BL = int(OFF[-1])                       # 7040 cols per (v, bl)
CNT_T = 2 * STRIP_BL                          # strip positions / partition / v
CNT_D = 2 * PAIRS_PER_BL * 8 * 16             # diag positions / partition / v

_PROG = None
TRACE = False
LAST_EXEC_NS = None
LAST_RESULTS = None


def _bn1_stats(v, w1):
    """Exact batch stats of einsum('bijc,oc->bijo', (v_i-v_j)^2, w1)."""
    Bv, Nv, _ = v.shape
    S1 = v.sum(1)
    S2 = (v ** 2).sum(1)
    P = np.einsum('bic,bid->bcd', v, v)
    Q = np.einsum('bic,bid->bcd', v ** 2, v)
    R = np.einsum('bic,bid->bcd', v ** 2, v ** 2)
    sim_sum = 2 * Nv * S2 - 2 * S1 ** 2
    M = (2 * Nv * R
         + 2 * np.einsum('bc,bd->bcd', S2, S2)
         + 4 * P ** 2
         - 4 * np.einsum('bcd,bd->bcd', Q, S1)
         - 4 * np.einsum('bdc,bc->bcd', Q, S1))
    n = Bv * Nv * Nv
    m1 = w1 @ (sim_sum.sum(0) / n)
    E2 = np.einsum('oc,cd,od->o', w1, M.sum(0) / n, w1)
    return m1, E2 - m1 ** 2


def build_program(n_cores=N_CORES, no_collective=False):
    nc = bacc.Bacc(None, target_bir_lowering=False, debug=False)

    def inp(name, shape, dt=F32):
        return nc.dram_tensor(name, list(shape), dt, kind="ExternalInput")

    VKS = ("mid", "pt")
    vshift = {vk: inp(f"vshift_{vk}", (128, BL, N)) for vk in VKS}
    vdup = {vk: inp(f"vdup_{vk}", (128, BL, N)) for vk in VKS}
    w1T = inp("w1T", (128, CH1), F16)
    w2T = inp("w2T", (CH1, C), F16)
    w3pair = inp("w3pair", (CH1, 2), F16)
    ab1 = {vk: inp(f"ab1_{vk}", (CH1, 2)) for vk in VKS}
    dsum = {vk: inp(f"dsum_{vk}", (128, 2)) for vk in VKS}
    g2b2 = inp("g2b2", (C, 2))
    b3p = inp("b3p", (128, 1))
    point_edge = inp("point_edge", (BL, N, N))
    dnT0 = inp("dnT0", (S, BL, N))
    p2d_wa = inp("p2d_wa", (S, G, S))
    p2d_wb = inp("p2d_wb", (S, G, S))
    p2d_bias = inp("p2d_bias", (S, G))
    maskdiag = inp("maskdiag", (N, N))
    eyeplus = inp("eyeplus", (N, N))
    ident = inp("ident", (128, 128))
    identh = inp("identh", (128, 128), F16)
    bdmask = inp("bdmask", (N, N), F16)

    out_dn = [nc.dram_tensor(f"out{l}", [BL, N, S], F32, kind="ExternalOutput")
              for l in range(G)]

    e_buf = {vk: nc.dram_tensor(f"ebuf_{vk}", [BL, N, N], F16) for vk in VKS}
    cc_in = {vk: nc.dram_tensor(f"ccin_{vk}", [128, 2], F32) for vk in VKS}
    cc_out = {vk: nc.dram_tensor(f"ccout_{vk}", [128, 2], F32,
                                 addr_space="Shared") for vk in VKS}
    groups = [list(range(n_cores))]

    with tile.TileContext(nc) as tc, \
         tc.tile_pool(name="singles", bufs=1) as singles, \
         tc.tile_pool(name="hpool", bufs=1) as hpool, \
         tc.tile_pool(name="wp", bufs=3) as wp, \
         tc.tile_pool(name="pmm", bufs=1, space="PSUM") as pmm, \
         tc.tile_pool(name="pe_ps", bufs=1, space="PSUM") as ppe, \
         tc.tile_pool(name="pc_ps", bufs=1, space="PSUM") as pps:

        dma = nc.default_dma_engine

        def load(t, shape, dt=F32, tag=None):
            sb = singles.tile(list(shape), dt, tag=tag or t.name,
                              name=tag or t.name)
            dma.dma_start(out=sb, in_=t[tuple(slice(0, s) for s in shape)])
            return sb

        vshift_sb = {vk: load(vshift[vk], (128, BL, N)) for vk in VKS}
        vdup_sb = {vk: load(vdup[vk], (128, BL, N)) for vk in VKS}
        w1T_sb = load(w1T, (128, CH1), F16)
        w2T_sb = load(w2T, (CH1, C), F16)
        w3p_sb = load(w3pair, (CH1, 2), F16)
        ab1_sb = {vk: load(ab1[vk], (CH1, 2)) for vk in VKS}
        dsum_sb = {vk: load(dsum[vk], (128, 2)) for vk in VKS}
        g2b2_sb = load(g2b2, (C, 2))
        b3_sb = load(b3p, (128, 1))
        dnT0_sb = load(dnT0, (S, BL, N))
        p2dwa_sb = load(p2d_wa, (S, G, S))
        p2dwb_sb = load(p2d_wb, (S, G, S))
        p2db_sb = load(p2d_bias, (S, G))
        ident_sb = load(ident, (128, 128))
        identh_sb = load(identh, (128, 128), F16)
        mask_sb = [load(maskdiag, (128, N), tag="mask0"),
                   singles.tile([32, N], F32, tag="mask1", name="mask1")]
        dma.dma_start(out=mask_sb[1], in_=maskdiag[128:160, :])
        eyep_sb = [load(eyeplus, (128, N), tag="eyep0"),
                   singles.tile([32, N], F32, tag="eyep1", name="eyep1")]
        dma.dma_start(out=eyep_sb[1], in_=eyeplus[128:160, :])
        bdm_sb = [load(bdmask, (128, N), F16, tag="bdm0"),
                  singles.tile([32, N], F16, tag="bdm1", name="bdm1")]
        dma.dma_start(out=bdm_sb[1], in_=bdmask[128:160, :])

        # zero-fill e_buf (one zero tile streamed out 4x)
        zsb = singles.tile([32, 800], F16, tag="zsb", name="zsb")
        nc.vector.memset(zsb, 0.0)
        for vk in VKS:
            ezv = e_buf[vk].rearrange("l (g r) c -> l g (r c)", g=32)
            for bl in range(BL):
                dma.dma_start(out=ezv[bl], in_=zsb)

        # resident h strips, f16: 128 x (4 * 7040) x 2B = 113 KiB/partition
        h_all = hpool.tile([128, 2, 2, 2, STRIP_BL], F16, tag="h_all")
        # stats: strips (T) and diag blocks (D), 18 subs per (bl)
        stats_T = {vk: singles.tile([128, 2 * NSUB_BL, 6], F32,
                                    tag=f"statsT_{vk}", name=f"statsT_{vk}")
                   for vk in VKS}

        def vki(vk):
            return 0 if vk == "mid" else 1

        # ---------------- pass A ----------------
        def pass_a(vk):
            for bl in range(BL):
                for p in range(PAIRS_PER_BL):
                    W = WP[p]
                    FW = 8 * W
                    a_sh = vshift_sb[vk][:, bl, 16 * p:16 * p + 8]
                    in0 = a_sh.unsqueeze(-1).broadcast_to([128, 8, W])
                    in1 = (vdup_sb[vk][:, bl, 16 * p:]
                           .unsqueeze(1).broadcast_to([128, 8, W]))
                    simtmp = wp.tile([128, 8 * N], F16, tag="simtmp")
                    stv = simtmp[:, :FW].rearrange("q (a b) -> q a b", b=W)
                    nc.gpsimd.tensor_sub(stv, in0, in1)
                    sim = wp.tile([128, 8 * N], F16, tag="sim")
                    nc.vector.tensor_mul(sim[:, :FW], simtmp[:, :FW],
                                         simtmp[:, :FW])

                    hAB = h_all[:, vki(vk), bl, :,
                                int(OFF[p]):int(OFF[p]) + FW]
                    for half in (0, 1):
                        rows = sim[64 * half:64 * half + 64, :]
                        for (i0, r) in RS[p]:
                            c0, c1 = i0 * W, (i0 + r) * W
                            h1 = pmm.tile([128, 512], F32, tag="h1", bufs=3)
                            nc.tensor.matmul(
                                h1[:, :c1 - c0],
                                lhsT=w1T_sb[64 * half:64 * half + 64, :],
                                rhs=rows[:, c0:c1],
                                start=True, stop=True)
                            nc.scalar.activation(
                                out=hAB[half, c0:c1], in_=h1[:, :c1 - c0],
                                func=AF.Prelu,
                                bias=ab1_sb[vk][:, 1:2],
                                scale=ab1_sb[vk][:, 0:1],
                                alpha=SLOPE)

                    sidx = sum(len(RS[q]) for q in range(p)) + bl * NSUB_BL
                    for si, (i0, r) in enumerate(RS[p]):
                        c0, c1 = i0 * W, (i0 + r) * W
                        h2 = pmm.tile([128, 512], F32, tag="h2", bufs=2)
                        nc.tensor.matmul(h2[0:64, :c1 - c0], lhsT=w2T_sb,
                                         rhs=hAB[0, c0:c1],
                                         start=True, stop=True)
                        nc.tensor.matmul(h2[64:128, :c1 - c0], lhsT=w2T_sb,
                                         rhs=hAB[1, c0:c1],
                                         start=True, stop=True)
                        nc.vector.bn_stats(
                            out=stats_T[vk][:, sidx + si, :],
                            in_=h2[:, :c1 - c0])

        # ------------- stats reduce + AllReduce -------------
        # global per-partition sums over ALL positions = 2*T - D
        def reduce_and_allreduce(vk):
            def agg(stats, cnt, tag):
                mv = singles.tile([128, 2], F32, tag=f"mv{tag}_{vk}",
                                  name=f"mv{tag}_{vk}")
                nc.vector.bn_aggr(out=mv, in_=stats)
                sm = singles.tile([128, 2], F32, tag=f"sm{tag}_{vk}",
                                  name=f"sm{tag}_{vk}")
                nc.vector.tensor_scalar_mul(sm[:, 0:1], mv[:, 0:1],
                                            float(cnt))
                msq = singles.tile([128, 1], F32, tag=f"mq{tag}_{vk}",
                                   name=f"mq{tag}_{vk}")
                nc.vector.tensor_mul(msq, mv[:, 0:1], mv[:, 0:1])
                nc.vector.tensor_add(msq, msq, mv[:, 1:2])
                nc.vector.tensor_scalar_mul(sm[:, 1:2], msq, float(cnt))
                return sm
            smT = agg(stats_T[vk], CNT_T, "T")
            sums = singles.tile([128, 2], F32, tag=f"sums_{vk}",
                                name=f"sums_{vk}")
            nc.vector.tensor_scalar_mul(sums, smT, 2.0)
            nc.vector.tensor_sub(sums, sums, dsum_sb[vk])
            dma.dma_start(out=cc_in[vk][:, :], in_=sums)
            if no_collective:
                dma.dma_start(out=cc_out[vk][:, :], in_=cc_in[vk][:, :])
            else:
                nc.gpsimd.collective_compute(
                    "AllReduce", ALU.add, replica_groups=groups,
                    ins=[cc_in[vk][:, :]], outs=[cc_out[vk][:, :]])

        # ------------- alpha2 / beta2 -------------
        def compute_ab2(vk):
            sp = wp
            gs = sp.tile([128, 2], F32, tag="gs")
            dma.dma_start(out=gs, in_=cc_out[vk][:, :])
            bot = sp.tile([C, 2], F32, tag="bot")
            dma.dma_start(out=bot, in_=gs[64:128, :])
            tot = sp.tile([C, 2], F32, tag="tot")
            nc.vector.tensor_add(tot, gs[0:64, :], bot)
            mE = sp.tile([C, 2], F32, tag="mE")
            nc.vector.tensor_scalar_mul(mE, tot, 1.0 / NTOT)
            var2 = sp.tile([C, 1], F32, tag="var2")
            nc.vector.tensor_mul(var2, mE[:, 0:1], mE[:, 0:1])
            nc.vector.tensor_sub(var2, mE[:, 1:2], var2)
            xe = sp.tile([C, 1], F32, tag="xe")
            nc.vector.tensor_scalar_add(xe, var2, BN_EPS)
            sq = sp.tile([C, 1], F32, tag="sq")
            eps_t = sp.tile([C, 1], F32, tag="eps_t")
            nc.vector.memset(eps_t, 0.0)
            nc.scalar.activation(out=sq, in_=xe, func=AF.Sqrt, bias=eps_t)
            r0 = sp.tile([C, 1], F32, tag="r0")
            nc.vector.reciprocal(r0, sq)
            t1 = sp.tile([C, 1], F32, tag="t1")
            nc.vector.tensor_mul(t1, r0, r0)
            nc.vector.tensor_mul(t1, t1, xe)
            nc.vector.tensor_scalar(t1, t1, -0.5, 1.5, ALU.mult, ALU.add)
            nc.vector.tensor_mul(r0, r0, t1)
            ab2 = sp.tile([C, 2], F32, tag="ab2")
            nc.vector.tensor_mul(ab2[:, 0:1], r0, g2b2_sb[:, 0:1])
            t2 = sp.tile([C, 1], F32, tag="t2")
            nc.vector.tensor_mul(t2, mE[:, 0:1], ab2[:, 0:1])
            nc.vector.tensor_sub(ab2[:, 1:2], g2b2_sb[:, 1:2], t2)
            ab2p = singles.tile([128, 2], F32, tag=f"ab2p_{vk}",
                                name=f"ab2p_{vk}")
            dma.dma_start(out=ab2p[0:64, :], in_=ab2)
            dma.dma_start(out=ab2p[64:128, :], in_=ab2)
            return ab2p

        # ---------------- pass B ----------------
        # e strip packed in PSUM at partition base 32*si; drained with a
        # single Sigmoid (b3 folded) -> esb_all -> strided DMA to e_buf.
        def pass_b(vk, ab2p, bl):
            esb_all = wp.tile([66, PAIRS_PER_BL, 512], F16, tag="esb",
                              bufs=1)
            ebvw = e_buf[vk].rearrange("l (pp h i) j -> l pp h i j",
                                       pp=PAIRS_PER_BL, h=2, i=8)
            for p in range(PAIRS_PER_BL):
                W = WP[p]
                FW = 8 * W
                hAB = h_all[:, vki(vk), bl, :,
                            int(OFF[p]):int(OFF[p]) + FW]
                eps_t = ppe.tile([66, 512], F32, tag="e_psum")
                maxw = 0
                for si, (i0, r) in enumerate(RS[p]):
                    c0, c1 = i0 * W, (i0 + r) * W
                    h2 = pmm.tile([128, 512], F32, tag="h2", bufs=2)
                    nc.tensor.matmul(h2[0:64, :c1 - c0], lhsT=w2T_sb,
                                     rhs=hAB[0, c0:c1],
                                     start=True, stop=True)
                    nc.tensor.matmul(h2[64:128, :c1 - c0], lhsT=w2T_sb,
                                     rhs=hAB[1, c0:c1],
                                     start=True, stop=True)
                    hh = wp.tile([128, 512], F16, tag="hh", bufs=3)
                    nc.scalar.activation(
                        out=hh[:, :c1 - c0], in_=h2[:, :c1 - c0],
                        func=AF.Prelu, bias=ab2p[:, 1:2],
                        scale=ab2p[:, 0:1], alpha=SLOPE)
                    nc.tensor.matmul(eps_t[32 * si:32 * si + 2, 0:c1 - c0],
                                     lhsT=w3p_sb, rhs=hh[:, :c1 - c0],
                                     start=True, stop=True)
                    maxw = max(maxw, c1 - c0)
                nc.scalar.activation(out=esb_all[:, p, :maxw],
                                     in_=eps_t[:, :maxw], func=AF.Sigmoid,
                                     bias=b3_sb[0:66])
                for si, (i0, r) in enumerate(RS[p]):
                    src = esb_all[32 * si:32 * si + 2, p, 0:r * W].rearrange(
                        "h (a b) -> h a b", b=W)
                    nc.gpsimd.dma_start(
                        out=ebvw[bl, p, :, i0:i0 + r, 16 * p:16 * p + W],
                        in_=src)

        # ---------------- e reconstruction:  e = U + U^T - U*blockdiag ----
        def recon(vk, bl, keep):
            U0 = keep.tile([128, N], F16, tag=f"U0_{vk}_{bl}")
            dma.dma_start(out=U0, in_=e_buf[vk][bl, 0:128, :])
            U1 = keep.tile([32, N], F16, tag=f"U1_{vk}_{bl}")
            dma.dma_start(out=U1, in_=e_buf[vk][bl, 128:160, :])
            m0 = keep.tile([128, N], F16, tag="m0")
            nc.vector.tensor_mul(m0, U0, bdm_sb[0])
            m1 = keep.tile([32, N], F16, tag="m1")
            nc.vector.tensor_mul(m1, U1, bdm_sb[1])
            tpb = pps.tile([128, 2 * N], F16, tag="tpc")
            tp0 = tpb[:, 0:N]
            tp1 = tpb[0:32, N:2 * N]
            nc.tensor.matmul(tp0[:, 0:128], lhsT=U0[:, 0:128],
                             rhs=identh_sb, is_transpose=True,
                             start=True, stop=True)
            nc.tensor.matmul(tp0[:, 128:160], lhsT=U1[:, 0:128],
                             rhs=identh_sb[0:32, 0:32], is_transpose=True,
                             start=True, stop=True)
            nc.tensor.matmul(tp1[:, 0:128], lhsT=U0[:, 128:160],
                             rhs=identh_sb, is_transpose=True,
                             start=True, stop=True)
            nc.tensor.matmul(tp1[:, 128:160], lhsT=U1[:, 128:160],
                             rhs=identh_sb[0:32, 0:32], is_transpose=True,
                             start=True, stop=True)
            e0 = keep.tile([128, N], F32, tag=f"e0_{vk}_{bl}")
            nc.vector.tensor_add(e0, tp0, m0)
            e1 = keep.tile([32, N], F32, tag=f"e1_{vk}_{bl}")
            nc.vector.tensor_add(e1, tp1, m1)
            return [e0, e1]

        # ---------------- epilogue ----------------
        def epilogue(pool, e_tiles, ep_tiles, tag):
            pe_tiles = []
            for blk, pdim in ((0, 128), (1, 32)):
                e_t, ep_t = e_tiles[blk], ep_tiles[blk]
                epm = pool.tile([pdim, N], F32, tag=f"epm{blk}")
                nc.vector.tensor_mul(epm, ep_t, mask_sb[blk][:pdim, :])
                rs = pool.tile([pdim, 1], F32, tag=f"rs{blk}")
                nc.vector.reduce_sum(rs, epm, axis=AX.X)
                x = pool.tile([pdim, N], F32, tag=f"x{blk}_{tag}")
                nc.vector.tensor_mul(x, e_t, epm)
                xs = pool.tile([pdim, 1], F32, tag=f"xs{blk}")
                nc.vector.reduce_sum(xs, x, axis=AX.X)
                nc.vector.tensor_scalar_max(xs, xs, 1e-12)
                rxs = pool.tile([pdim, 1], F32, tag=f"rxs{blk}")
                nc.vector.reciprocal(rxs, xs)
                nc.vector.tensor_mul(rxs, rxs, rs)
                nc.vector.tensor_scalar_mul(x, x, rxs)
                nc.vector.tensor_add(x, x, eyep_sb[blk][:pdim, :])
                rs2 = pool.tile([pdim, 1], F32, tag=f"rs2{blk}")
                nc.vector.reduce_sum(rs2, x, axis=AX.X)
                rrs2 = pool.tile([pdim, 1], F32, tag=f"rrs2{blk}")
                nc.vector.reciprocal(rrs2, rs2)
                nc.vector.tensor_scalar_mul(x, x, rrs2)
                pe_tiles.append(x)
            return pe_tiles

        def pe_transpose(pool, psum_pool, src_ap, pdim, fdim, tag="tp"):
            ps = psum_pool.tile([128, N], F32, tag="pc")
            nc.tensor.matmul(ps[:fdim, :pdim], lhsT=src_ap,
                             rhs=ident_sb[:pdim, :pdim],
                             is_transpose=True, start=True, stop=True)
            dst = pool.tile([fdim, pdim], F32, tag=f"tps{fdim}_{pdim}")
            nc.vector.tensor_copy(dst, ps[:fdim, :pdim])
            return dst

        # ---------------- phase C ----------------
        pe1_tiles = {}

        def phase_c1(bl, pool, keep):
            e_mid = recon("mid", bl, keep)
            ep00 = keep.tile([128, N], F32, tag=f"ep00_{bl}")
            dma.dma_start(out=ep00, in_=point_edge[bl, 0:128, :])
            ep01 = keep.tile([32, N], F32, tag=f"ep01_{bl}")
            dma.dma_start(out=ep01, in_=point_edge[bl, 128:160, :])
            pe1_tiles[bl] = epilogue(pool, e_mid, [ep00, ep01], f"pe1_{bl}")

        def phase_c2(bl, pool, keep, pps):
            e_pt = recon("pt", bl, keep)
            pe2 = epilogue(pool, e_pt, pe1_tiles[bl], f"pe2_{bl}")
            pe3 = epilogue(pool, e_pt, pe2, f"pe3_{bl}")
            dn_prev = None
            for l, pe_t in ((0, pe2), (1, pe3)):
                xT = keep.tile([S, N], F32, tag=f"xT_{bl}_{l}")
                t0 = pe_transpose(pool, pps, pe_t[0][:, 0:S], 128, S)
                nc.vector.tensor_copy(xT[:, 0:128], t0)
                t1 = pe_transpose(pool, pps, pe_t[1][:, 0:S], 32, S)
                nc.vector.tensor_copy(xT[:, 128:160], t1)
                dnT = dnT0_sb[:, bl, :] if l == 0 else dn_prev
                mm = pps.tile([128, N], F32, tag="pc")
                nc.tensor.matmul(mm[:S, :], lhsT=p2dwa_sb[:, l, :], rhs=xT,
                                 start=True, stop=False)
                nc.tensor.matmul(mm[:S, :], lhsT=p2dwb_sb[:, l, :], rhs=dnT,
                                 start=False, stop=True)
                dn_new = keep.tile([S, N], F32, tag=f"dnT_{bl}_{l}")
                nc.scalar.activation(out=dn_new, in_=mm[:S, :],
                                     func=AF.Prelu,
                                     bias=p2db_sb[:, l:l + 1],
                                     alpha=SLOPE)
                dn_prev = dn_new
                o0 = pe_transpose(pool, pps, dn_new[:, 0:128], S, 128)
                dma.dma_start(out=out_dn[l][bl, 0:128, :], in_=o0)
                o1 = pe_transpose(pool, pps, dn_new[:, 128:160], S, 32)
                dma.dma_start(out=out_dn[l][bl, 128:160, :], in_=o1)

        # ---------------- schedule ----------------
        pass_a("mid")
        reduce_and_allreduce("mid")
        pass_a("pt")
        reduce_and_allreduce("pt")
        ab2p_mid = compute_ab2("mid")
        with tc.tile_pool(name="pc", bufs=2) as pool, \
             tc.tile_pool(name="pc_keep", bufs=1) as keep:
            pass_b("mid", ab2p_mid, 0)
            pass_b("mid", ab2p_mid, 1)
            phase_c1(0, pool, keep)
            phase_c1(1, pool, keep)
            ab2p_pt = compute_ab2("pt")
            pass_b("pt", ab2p_pt, 0)
            phase_c2(0, pool, keep, pps)
            pass_b("pt", ab2p_pt, 1)
            phase_c2(1, pool, keep, pps)

    nc.compile()
    return nc


def _prep_maps(middle_node, point_node, distribution_node, distribution_edge,
               point_edge, w1, g1, b1, w2, g2, b2, w3, b3, p2d_w, p2d_b,
               n_cores=N_CORES):
    f4 = np.float32
    middle_node = np.asarray(middle_node)
    point_node = np.asarray(point_node)

    def vt_pair(v_local):
        vT = np.transpose(v_local, (0, 2, 1)).astype(f4)      # [BL, C, N]
        sh = np.concatenate([vT[:, :, 8:], np.zeros((BL, C, 8), f4)], axis=2)
        vshift = np.concatenate([vT, sh], axis=1)             # [BL, 128, N]
        vdup = np.concatenate([vT, vT], axis=1)
        # -> [128, BL, N]
        return (np.ascontiguousarray(np.transpose(vshift, (1, 0, 2))),
                np.ascontiguousarray(np.transpose(vdup, (1, 0, 2))))

    def ab1_for(v):
        m1, var1 = _bn1_stats(v.astype(np.float64), np.asarray(w1, np.float64))
        a = np.asarray(g1, np.float64) / np.sqrt(var1 + BN_EPS)
        bb = np.asarray(b1, np.float64) - m1 * a
        return np.ascontiguousarray(np.stack([a, bb], axis=1).astype(f4))

    ab1_mid = ab1_for(middle_node)
    ab1_pt = ab1_for(point_node)

    w1_64 = np.asarray(w1, np.float64)
    w2_64 = np.asarray(w2, np.float64)

    def diag_sums(v_local, ab1_h):
        # exact fp64 sums of h2 = w2 @ lrelu(a1*(w1@sim)+b1) over the
        # 16x16 diagonal blocks, split per (channel, row-half) partition
        vb = np.asarray(v_local, np.float64).reshape(BL, PAIRS_PER_BL, 16,
                                                     C)
        sim = (vb[:, :, :, None, :] - vb[:, :, None, :, :]) ** 2
        h1 = sim @ w1_64.T                                   # [bl,p,i,j,128]
        a = ab1_h[:, 0].astype(np.float64)
        b = ab1_h[:, 1].astype(np.float64)
        z = a * h1 + b
        h = np.where(z >= 0, z, SLOPE * z)
        h2 = h @ w2_64.T                                     # [bl,p,i,j,64]
        out = np.zeros((128, 2), np.float32)
        for half in range(2):
            blk = h2[:, :, 8 * half:8 * half + 8, :, :]
            out[64 * half:64 * half + 64, 0] = blk.sum((0, 1, 2, 3))
            out[64 * half:64 * half + 64, 1] = (blk ** 2).sum((0, 1, 2, 3))
        return out

    w1T_h = np.ascontiguousarray(np.concatenate(
        [np.asarray(w1).T, np.asarray(w1).T], axis=0).astype(np.float16))
    w2T_h = np.ascontiguousarray(np.asarray(w2).T.astype(np.float16))
    w3pair_h = np.zeros((CH1, 2), np.float16)
    w3pair_h[0:C, 0] = np.asarray(w3).astype(np.float16)
    w3pair_h[C:CH1, 1] = np.asarray(w3).astype(np.float16)
    g2b2_h = np.ascontiguousarray(np.stack([np.asarray(g2), np.asarray(b2)],
                                           axis=1).astype(f4))
    b3p_h = np.full((128, 1), float(np.asarray(b3)), f4)
    pw = np.asarray(p2d_w)
    p2d_wa_h = np.ascontiguousarray(
        np.transpose(pw[:, :, 0:S], (2, 0, 1)).astype(f4))      # [S,G,S]
    p2d_wb_h = np.ascontiguousarray(
        np.transpose(pw[:, :, S:2 * S], (2, 0, 1)).astype(f4))  # [S,G,S]
    p2d_bias_h = np.ascontiguousarray(np.asarray(p2d_b).T.astype(f4))
    maskdiag_h = (1.0 - np.eye(N)).astype(f4)
    eyeplus_h = (np.eye(N) + 1e-6).astype(f4)
    ident_h = np.eye(128, dtype=f4)
    identh_h = np.eye(128, dtype=np.float16)
    bd = np.ones((N, N), np.float16)
    for p in range(N // 16):
        bd[16 * p:16 * p + 16, 16 * p:16 * p + 16] = 0.0

    maps = []
    for c in range(n_cores):
        sl = slice(c * BL, (c + 1) * BL)
        vs_m, vd_m = vt_pair(middle_node[sl])
        vs_p, vd_p = vt_pair(point_node[sl])
        ds_m = diag_sums(middle_node[sl], ab1_mid)
        ds_p = diag_sums(point_node[sl], ab1_pt)
        dnT0_h = np.ascontiguousarray(
            np.transpose(np.asarray(distribution_node)[sl], (2, 0, 1))
            .astype(f4))                                        # [S,BL,N]
        maps.append(dict(
            vshift_mid=vs_m, vdup_mid=vd_m, vshift_pt=vs_p, vdup_pt=vd_p,
            w1T=w1T_h, w2T=w2T_h, w3pair=w3pair_h,
            ab1_mid=ab1_mid, ab1_pt=ab1_pt, g2b2=g2b2_h, b3p=b3p_h,
            dsum_mid=ds_m, dsum_pt=ds_p,
            point_edge=np.ascontiguousarray(
                np.asarray(point_edge)[sl].astype(f4)),
            dnT0=dnT0_h, p2d_wa=p2d_wa_h, p2d_wb=p2d_wb_h,
            p2d_bias=p2d_bias_h,
            maskdiag=maskdiag_h, eyeplus=eyeplus_h, ident=ident_h,
            identh=identh_h, bdmask=bd,
        ))
    return maps


def kernel(**inputs):
    global _PROG, LAST_EXEC_NS, LAST_RESULTS
    if _PROG is None:
        _PROG = build_program()
    maps = _prep_maps(**inputs)
    res = run_bass_kernel_spmd(_PROG, maps, core_ids=list(range(N_CORES)),
                               trace=TRACE)
    LAST_EXEC_NS = res.exec_time_ns
    LAST_RESULTS = res
    outs = []
    for l in range(G):
        outs.append(np.concatenate([res.results[c][f"out{l}"]
                                    for c in range(N_CORES)], axis=0))
    return tuple(outs)
